# revision 1
# baseline (speedup 1.0000x reference)
"""Trainium2 Bass kernel for nn_Decoder_14680198217759.

Multi-head attention decoder (B=32, G=N=512, E=128, H=8, D=16), pure data
parallel over 8 NeuronCores (4 batches/core), fp32 throughout.

Layout strategy per batch (all on one core):
  - Activations transposed via PE so E sits on partitions: xT [E, G].
  - Projections produce qT/kT in a head-padded layout (head j of a 4-head
    group at partition offset 32j) so per-head K=16 score matmuls can use
    legal base partitions and 2-way PE row tiling.
  - Scores computed TRANSPOSED: scoreT_h [n, g] (head pair packed into one
    [128, 1024] PSUM tile), so the softmax(n) contraction of PV needs no
    transpose of the big prob matrix.
  - The rank-3 mask rides the PE: maskT is added into the score PSUM via an
    identity-matmul (lhsT = natural mask chunk, rhs = I) per head.
  - exp on ACT (PSUM -> SBUF). PV contracts exp with v_aug (v plus a ones
    column) giving out^T and the softmax denominators in one pass.
  - Per-head normalization happens in natural space: PV output transposed
    back [g, hd], reciprocal on the strided ones columns, per-partition
    tensor_scalar multiplies, transpose again for the combine matmul.
  - Branch 2 (single-head scoring) stays natural [g, n]: tanh on ACT with
    input scale 1/sqrt(E); mask added with one DVE tensor_tensor (the mask is
    scale invariant: 10*(t + m) masks as well as 10 t + m); exp with scale=10
    and accum_out producing the denominators for free.
"""

import numpy as np

B, G, N, E, H, D = 32, 512, 512, 128, 8, 16
SQRT_E = 11.313708498984761
NCORES = 8
BL = B // NCORES  # batches per core

_CACHE = {}


# --------------------------------------------------------------------------
# BIR wait legalization: this toolchain's walrus accepts at most ONE sem wait
# per instruction; Tile's scheduler can emit more (notably on the kernel-tail
# drain). Split excess waits onto same-engine NoOps placed directly before
# the offending instruction (same-queue program order keeps the semantics).
# --------------------------------------------------------------------------
def _legalize_waits(nc, max_waits=1):
    import concourse.mybir as mybir

    n_split = 0
    for f in nc.m.functions:
        for bb in f.blocks:
            out = []
            for ins in bb.instructions:
                si = ins.sync_info
                waits = list(si.on_wait) if si and si.on_wait else []
                if len(waits) > max_waits:
                    while len(waits) > max_waits:
                        chunk, waits = waits[:max_waits], waits[max_waits:]
                        nop = mybir.InstNoOp(
                            name=f"I-waitfix-{nc.next_id()}", ins=[], outs=[]
                        )
                        nop.engine = ins.engine
                        nop.sync_info = mybir.SyncInfo(on_wait=chunk, on_update=[])
                        out.append(nop)
                        n_split += 1
                    ins.sync_info = mybir.SyncInfo(
                        on_wait=waits, on_update=list(si.on_update or [])
                    )
                out.append(ins)
            bb.instructions[:] = out
    return n_split


def _build_nc(legalize=True):
    import concourse.bass as bass
    import concourse.mybir as mybir
    import concourse.tile as tile
    from concourse.masks import make_identity

    f32 = mybir.dt.float32
    f32r = mybir.dt.float32r  # PE fast fp32 path (1 cycle/row vs 4)
    bf16 = mybir.dt.bfloat16
    AF = mybir.ActivationFunctionType

    nc = bass.Bass()

    nodes_d = nc.dram_tensor("nodes", [BL, N, E], f32, kind="ExternalInput")
    q1_d = nc.dram_tensor("q1", [BL, G, E], f32, kind="ExternalInput")
    last_d = nc.dram_tensor("last", [BL, G, E], f32, kind="ExternalInput")
    mask_d = nc.dram_tensor("mask", [BL, G, N], f32, kind="ExternalInput")
    wnames = ["Wq1p0", "Wq1p1", "Wqlp0", "Wqlp1", "Wkp0", "Wkp1", "Wv", "Wc"]
    w_d = {n: nc.dram_tensor(n, [E, 128], f32, kind="ExternalInput") for n in wnames}
    b_d = nc.dram_tensor("bc", [E, 1], f32, kind="ExternalInput")
    probs_d = nc.dram_tensor("probs", [BL, G, N], f32, kind="ExternalOutput")

    with tile.TileContext(nc) as tc:
        import contextlib

        with contextlib.ExitStack() as ctx:
            pw = ctx.enter_context(tc.tile_pool(name="pw", bufs=1))
            pin = ctx.enter_context(tc.tile_pool(name="pin", bufs=3))
            pxt = ctx.enter_context(tc.tile_pool(name="pxt", bufs=3))
            pproj = ctx.enter_context(tc.tile_pool(name="pproj", bufs=3))
            pexp = ctx.enter_context(tc.tile_pool(name="pexp", bufs=6))
            pmisc = ctx.enter_context(tc.tile_pool(name="pmisc", bufs=3))
            pstage = ctx.enter_context(tc.tile_pool(name="pstage", bufs=3))
            ps_score = ctx.enter_context(
                tc.tile_pool(name="ps_score", bufs=2, space="PSUM")
            )
            ps_pv = ctx.enter_context(tc.tile_pool(name="ps_pv", bufs=2, space="PSUM"))
            ps_m = ctx.enter_context(tc.tile_pool(name="ps_m", bufs=2, space="PSUM"))

            # ---- constants / weights (once) ----
            ident = pw.tile([128, 128], f32)
            make_identity(nc, ident)
            ident_b = pw.tile([128, 128], bf16)
            make_identity(nc, ident_b)
            # weights: DMA raw fp32, then one-time round to f32r for the PE
            w_sb = {}
            for n in wnames:
                w_raw = pw.tile([128, 128], f32, name=f"wr_{n}", tag=f"wr_{n}")
                nc.sync.dma_start(out=w_raw, in_=w_d[n][:, :])
                w_sb[n] = pw.tile([128, 128], f32r, name=f"w_{n}", tag=f"w_{n}")
                nc.vector.tensor_copy(w_sb[n], w_raw)
            b_sb = pw.tile([128, 1], f32)
            nc.sync.dma_start(out=b_sb, in_=b_d[:, :])
            # v_aug: per n-chunk, 8 heads at 32-col blocks: cols 32h..32h+15 =
            # v head h, col 32h+16 = 1.0 (denominator row), rest zero.
            v_aug = pw.tile([128, 4, 256], bf16)
            nc.vector.memset(v_aug, 0.0)
            v_aug_blk = v_aug.rearrange("p c (h x) -> p c h x", x=32)
            nc.vector.memset(v_aug_blk[:, :, :, 16:17], 1.0)

            for b in range(BL):
                # ---- loads ----
                x_nodes = pin.tile([128, 4, 128], f32)
                nc.sync.dma_start(
                    out=x_nodes, in_=nodes_d[b].rearrange("(c p) e -> p c e", p=128)
                )
                x_q1 = pin.tile([128, 4, 128], f32)
                nc.sync.dma_start(
                    out=x_q1, in_=q1_d[b].rearrange("(c p) e -> p c e", p=128)
                )
                x_last = pin.tile([128, 4, 128], f32)
                nc.sync.dma_start(
                    out=x_last, in_=last_d[b].rearrange("(c p) e -> p c e", p=128)
                )
                mask_t = pin.tile([128, 4, 512], bf16)
                nc.gpsimd.dma_start(
                    out=mask_t, in_=mask_d[b].rearrange("(c p) n -> p c n", p=128)
                )

                # ---- transpose activations to [E, G] ----
                def transpose_to(dst, src_nat):
                    for c in range(4):
                        tp = ps_m.tile([128, 128], f32, tag="m")
                        nc.tensor.transpose(tp, src_nat[:, c, :], ident)
                        nc.vector.tensor_copy(dst[:, c * 128 : (c + 1) * 128], tp)

                nodesT = pxt.tile([128, 512], f32r)
                transpose_to(nodesT, x_nodes)
                q1T = pxt.tile([128, 512], f32r)
                transpose_to(q1T, x_q1)
                lastT = pxt.tile([128, 512], f32r)
                transpose_to(lastT, x_last)

                # ---- projections (outputs transposed/padded) ----
                qT = []
                for g4 in range(2):  # head groups 0-3 / 4-7
                    ps = ps_m.tile([128, 512], f32, tag="m")
                    nc.tensor.matmul(
                        ps, w_sb[f"Wq1p{g4}"], q1T, start=True, stop=False
                    )
                    nc.tensor.matmul(
                        ps, w_sb[f"Wqlp{g4}"], lastT, start=False, stop=True
                    )
                    t = pproj.tile([128, 512], f32r, tag=f"qT{g4}")
                    nc.vector.tensor_copy(t, ps)
                    qT.append(t)
                kT = []
                for g4 in range(2):
                    ps = ps_m.tile([128, 512], f32, tag="m")
                    nc.tensor.matmul(
                        ps, w_sb[f"Wkp{g4}"], nodesT, start=True, stop=True
                    )
                    t = pproj.tile([128, 512], f32r, tag=f"kT{g4}")
                    nc.vector.tensor_copy(t, ps)
                    kT.append(t)
                # v natural [n, hd] scattered into v_aug 32-blocks
                for c in range(4):
                    ps = ps_m.tile([128, 128], f32, tag="m")
                    nc.tensor.matmul(
                        ps,
                        nodesT[:, c * 128 : (c + 1) * 128],
                        w_sb["Wv"],
                        start=True,
                        stop=True,
                    )
                    nc.vector.tensor_copy(
                        v_aug_blk[:, c, :, 0:16],
                        ps.rearrange("p (h d) -> p h d", d=16),
                    )

                # ---- attention: scores (transposed) + exp + PV ----
                pv_banks = []
                for half in range(2):  # heads 0-3 then 4-7
                    pv = ps_pv.tile([128, 512], f32, tag="pv")
                    pv_banks.append(pv)
                    for hp in range(2):  # head pair within group
                        j0, j1 = 2 * hp, 2 * hp + 1  # pad-slot indices
                        expps = []
                        for c in range(4):  # n-chunks
                            sc = ps_score.tile([128, 1024], f32, tag="sc")
                            for idx, j in enumerate((j0, j1)):
                                nc.tensor.matmul(
                                    sc[:, idx * 512 : (idx + 1) * 512],
                                    kT[half][
                                        32 * j : 32 * j + 16,
                                        c * 128 : (c + 1) * 128,
                                    ],
                                    qT[half][32 * j : 32 * j + 16, :],
                                    start=True,
                                    stop=False,
                                    tile_position=(32 * j, 0),
                                )
                            # += maskT via identity trick, per head.
                            # gc outer so consecutive MMs share the same
                            # stationary mask chunk (one weight load, 2 MMs).
                            for gc in range(4):
                                for idx in range(2):
                                    nc.tensor.matmul(
                                        sc[
                                            :,
                                            idx * 512
                                            + gc * 128 : idx * 512
                                            + (gc + 1) * 128,
                                        ],
                                        mask_t[:, gc, c * 128 : (c + 1) * 128],
                                        ident_b,
                                        start=False,
                                        stop=(gc == 3),
                                    )
                            expp = pexp.tile([128, 1024], bf16, tag="expp")
                            nc.scalar.activation(expp, sc, AF.Exp)
                            expps.append(expp)
                        # PV accumulate: out_aug^T rows 32j. The psum group
                        # checker requires groups in one bank to be strictly
                        # sequential, so each head's 4 n-chunk MMs run
                        # back-to-back as one start..stop group.
                        for idx, j in enumerate((j0, j1)):
                            vcol = 32 * (half * 4 + j)  # head's v_aug block
                            for c in range(4):
                                nc.tensor.matmul(
                                    pv[32 * j : 32 * j + 32, :],
                                    v_aug[:, c, vcol : vcol + 32],
                                    expps[c][:, idx * 512 : (idx + 1) * 512],
                                    start=(c == 0),
                                    stop=(c == 3),
                                    tile_position=(0, 32 * j),
                                )

                # ---- normalize per head, in natural space ----
                u_nat = pmisc.tile([128, 4, 256], f32)
                for half in range(2):
                    uT_pad = pmisc.tile([128, 512], f32, tag="uT_pad")
                    nc.vector.tensor_copy(uT_pad, pv_banks[half])
                    for gc in range(4):
                        tp = ps_m.tile([128, 128], f32, tag="m")
                        nc.tensor.transpose(
                            tp, uT_pad[:, gc * 128 : (gc + 1) * 128], ident
                        )
                        nc.vector.tensor_copy(
                            u_nat[:, gc, half * 128 : (half + 1) * 128], tp
                        )
                u_blk = u_nat.rearrange("p c (h x) -> p c h x", x=32)
                recip8 = pmisc.tile([128, 4, 8, 1], f32)
                nc.vector.reciprocal(recip8, u_blk[:, :, :, 16:17])
                u_c = pmisc.tile([128, 4, 128], f32)
                for gc in range(4):
                    for h in range(8):
                        nc.vector.tensor_scalar_mul(
                            u_c[:, gc, 16 * h : 16 * h + 16],
                            u_blk[:, gc, h, 0:16],
                            recip8[:, gc, h, :],
                        )
                uT_norm = pmisc.tile([128, 512], f32r)
                for gc in range(4):
                    tp = ps_m.tile([128, 128], f32, tag="m")
                    nc.tensor.transpose(tp, u_c[:, gc, :], ident)
                    nc.vector.tensor_copy(uT_norm[:, gc * 128 : (gc + 1) * 128], tp)

                # ---- combine + bias ----
                mh_ps = ps_m.tile([128, 512], f32, tag="m")
                nc.tensor.matmul(
                    mh_ps, w_sb["Wc"], uT_norm, start=True, stop=True
                )
                mhT = pmisc.tile([128, 512], f32r)
                nc.vector.tensor_scalar_add(mhT, mh_ps, b_sb[:, 0:1])

                # ---- branch 2: single-head scoring, natural layout ----
                tanh_sb = pstage.tile([128, 4, 512], f32)
                for gc in range(4):
                    s2 = ps_m.tile([128, 512], f32, tag="m")
                    nc.tensor.matmul(
                        s2,
                        mhT[:, gc * 128 : (gc + 1) * 128],
                        nodesT,
                        start=True,
                        stop=True,
                    )
                    nc.scalar.activation(
                        tanh_sb[:, gc, :], s2, AF.Tanh, scale=1.0 / SQRT_E
                    )
                # z = tanh + mask (mask scale-invariant under the x10)
                nc.gpsimd.tensor_add(tanh_sb, tanh_sb, mask_t)
                den = pmisc.tile([128, 4], f32)
                p_sb = pstage.tile([128, 4, 512], f32)
                for gc in range(4):
                    nc.scalar.activation(
                        p_sb[:, gc, :],
                        tanh_sb[:, gc, :],
                        AF.Exp,
                        scale=10.0,
                        accum_out=den[:, gc : gc + 1],
                    )
                recipden = pmisc.tile([128, 4], f32)
                nc.vector.reciprocal(recipden, den)
                stage = pstage.tile([128, 4, 512], f32)
                for gc in range(4):
                    nc.vector.tensor_scalar_mul(
                        stage[:, gc, :], p_sb[:, gc, :], recipden[:, gc : gc + 1]
                    )
                nc.sync.dma_start(
                    out=probs_d[b].rearrange("(c p) n -> p c n", p=128), in_=stage
                )

    if legalize:
        _legalize_waits(nc)
    return nc


def _prep_weights(inputs):
    def pad4(W):
        Wp0 = np.zeros((E, 128), np.float32)
        Wp1 = np.zeros((E, 128), np.float32)
        for j in range(4):
            Wp0[:, 32 * j : 32 * j + 16] = W[:, 16 * j : 16 * j + 16]
            Wp1[:, 32 * j : 32 * j + 16] = W[:, 64 + 16 * j : 64 + 16 * j + 16]
        return Wp0, Wp1

    s = np.float32(1.0 / np.sqrt(np.float32(D)))  # 0.25 folded into q weights
    Wq1p0, Wq1p1 = pad4(np.asarray(inputs["Wq_first"], np.float32) * s)
    Wqlp0, Wqlp1 = pad4(np.asarray(inputs["Wq_last"], np.float32) * s)
    Wkp0, Wkp1 = pad4(np.asarray(inputs["Wk"], np.float32))
    return {
        "Wq1p0": Wq1p0,
        "Wq1p1": Wq1p1,
        "Wqlp0": Wqlp0,
        "Wqlp1": Wqlp1,
        "Wkp0": Wkp0,
        "Wkp1": Wkp1,
        "Wv": np.ascontiguousarray(np.asarray(inputs["Wv"], np.float32)),
        "Wc": np.ascontiguousarray(np.asarray(inputs["W_comb"], np.float32)),
        "bc": np.asarray(inputs["b_comb"], np.float32).reshape(E, 1),
    }


def run(inputs, trace=False):
    from concourse.bass_utils import run_bass_kernel_spmd

    if "nc" not in _CACHE:
        _CACHE["nc"] = _build_nc()
    nc = _CACHE["nc"]

    w = _prep_weights(inputs)
    nodes = np.ascontiguousarray(np.asarray(inputs["encoded_nodes"], np.float32))
    q1 = np.ascontiguousarray(np.asarray(inputs["encoded_q1"], np.float32))
    last = np.ascontiguousarray(np.asarray(inputs["encoded_last_node"], np.float32))
    mask = np.ascontiguousarray(np.asarray(inputs["ninf_mask"], np.float32))

    in_maps = []
    for i in range(NCORES):
        sl = slice(i * BL, (i + 1) * BL)
        in_maps.append(
            {
                "nodes": nodes[sl],
                "q1": q1[sl],
                "last": last[sl],
                "mask": mask[sl],
                **w,
            }
        )
    try:
        res = run_bass_kernel_spmd(nc, in_maps, list(range(NCORES)), trace=trace)
    except Exception:
        # The first execution of a freshly compiled NEFF occasionally dies
        # with NRT_EXEC_UNIT_UNRECOVERABLE on this stack; a retry with the
        # cached NEFF has always succeeded.
        res = run_bass_kernel_spmd(nc, in_maps, list(range(NCORES)), trace=trace)
    out = np.concatenate([res.results[i]["probs"] for i in range(NCORES)], axis=0)
    return out, res


def kernel(**inputs) -> np.ndarray:
    out, _ = run(inputs, trace=False)
    return out



# revision 34
# speedup vs baseline: 1.5995x; 1.5995x over previous
"""Trainium2 Bass kernel for nn_Decoder_14680198217759.

Multi-head attention decoder (B=32, G=N=512, E=128, H=8, D=16), pure data
parallel over 8 NeuronCores (4 batches/core).

Layout strategy per batch (all on one core):
  - Activations transposed via PE (bf16 identity => 1 cycle/row, exact
    permutation path) so E sits on partitions: xT [E, G] in f32r.
  - Q/K projections produce qT/kT head-padded (head j of a 4-head group at
    partition offset 32j) so per-head K=16 score matmuls use legal base
    partitions via tile_position.
  - Scores TRANSPOSED: scoreT_h [n, g], head pair packed in one [128,1024]
    PSUM tile.  The rank-3 mask rides the PE as an fp8 DoubleRow matmul:
    stationary S[k,i,m] = (m==2k+i) expands a pair-packed fp8 mask (0/-240)
    moving operand at 0.5 cycles/row -- exact, half the cost of the bf16
    identity trick.  exp(s-240) underflows to exactly 0 in fp32, matching
    the reference's -1e9 mask.
  - exp on ACT (PSUM -> SBUF, bf16).  PV contracts exp with v_aug (v plus a
    ones column per 32-block) giving out^T plus softmax denominators.
  - Normalization stays transposed: reciprocal of the strided den rows, a
    [4,128] 0/1 broadcast matmul expands 1/den across each head's 16 rows,
    one DVE multiply produces u_norm; combine uses padded W_comb halves.
  - Branch 2 (single-head scoring) natural [g, n]: tanh (scale 1/sqrt(E))
    to fp16, fp16 mask/10 added on DVE (2x mode), exp with scale=10 and
    accum_out denominators, 4x-mode tensor_scalar scale to bf16, casting
    DMA (gpsimd) upconverts to the f32 output.
"""

import numpy as np

B, G, N, E, H, D = 32, 512, 512, 128, 8, 16
SQRT_E = 11.313708498984761
NCORES = 8
BL = B // NCORES  # batches per core

_CACHE = {}


# --------------------------------------------------------------------------
# BIR wait legalization: this toolchain's walrus accepts at most ONE sem wait
# per instruction; Tile's scheduler can emit more (notably on the kernel-tail
# drain). Split excess waits onto same-engine NoOps placed directly before
# the offending instruction (same-queue program order keeps the semantics).
# --------------------------------------------------------------------------
def _legalize_waits(nc, max_waits=1):
    import concourse.mybir as mybir

    n_split = 0
    for f in nc.m.functions:
        for bb in f.blocks:
            out = []
            for ins in bb.instructions:
                si = ins.sync_info
                waits = list(si.on_wait) if si and si.on_wait else []
                if len(waits) > max_waits:
                    while len(waits) > max_waits:
                        chunk, waits = waits[:max_waits], waits[max_waits:]
                        nop = mybir.InstNoOp(
                            name=f"I-waitfix-{nc.next_id()}", ins=[], outs=[]
                        )
                        nop.engine = ins.engine
                        nop.sync_info = mybir.SyncInfo(on_wait=chunk, on_update=[])
                        out.append(nop)
                        n_split += 1
                    ins.sync_info = mybir.SyncInfo(
                        on_wait=waits, on_update=list(si.on_update or [])
                    )
                out.append(ins)
            bb.instructions[:] = out
    return n_split


def _build_nc(legalize=True):
    import concourse.bass as bass
    import concourse.mybir as mybir
    import concourse.tile as tile
    from concourse.masks import make_identity

    f32 = mybir.dt.float32
    f32r = mybir.dt.float32r
    bf16 = mybir.dt.bfloat16
    fp16 = mybir.dt.float16
    f8 = mybir.dt.float8e4
    AF = mybir.ActivationFunctionType
    DR = mybir.MatmulPerfMode.DoubleRow

    nc = bass.Bass()

    nodes_d = nc.dram_tensor("nodes", [BL, N, E], fp16, kind="ExternalInput")
    q1_d = nc.dram_tensor("q1", [BL, G, E], fp16, kind="ExternalInput")
    last_d = nc.dram_tensor("last", [BL, G, E], fp16, kind="ExternalInput")
    # pair-packed transposed fp8 mask: mdr[b, k, c, i, g] = M240[b, g, 128c+2k+i]
    mdr_d = nc.dram_tensor("mdr", [BL, 64, 4, 2, G], f8, kind="ExternalInput")
    # natural fp16 mask / 10 (branch2 exp runs with scale=10)
    m16_d = nc.dram_tensor("m16", [BL, G, N], fp16, kind="ExternalInput")
    # 8 padded f32r weights combined into one DMA: [E, i, 128]
    WNAMES = ["Wq1p0", "Wq1p1", "Wqlp0", "Wqlp1", "Wkp0", "Wkp1", "Wcp0", "Wcp1"]
    wall_d = nc.dram_tensor("wall", [E, 8, 128], fp16, kind="ExternalInput")
    wv_d = nc.dram_tensor("Wv", [E, 128], fp16, kind="ExternalInput")
    sdr_d = nc.dram_tensor("Sdr", [64, 256], f8, kind="ExternalInput")
    b4_d = nc.dram_tensor("B4", [4, 128], fp16, kind="ExternalInput")
    b_d = nc.dram_tensor("bc", [E, 1], f32, kind="ExternalInput")
    probs_d = nc.dram_tensor("probs", [BL, G, N], f32, kind="ExternalOutput")

    with tile.TileContext(nc) as tc:
        import contextlib

        with contextlib.ExitStack() as ctx:
            pw = ctx.enter_context(tc.tile_pool(name="pw", bufs=1))
            pin = ctx.enter_context(tc.tile_pool(name="pin", bufs=3))
            pmask = ctx.enter_context(tc.tile_pool(name="pmask", bufs=3))
            pxt = ctx.enter_context(tc.tile_pool(name="pxt", bufs=3))
            pproj = ctx.enter_context(tc.tile_pool(name="pproj", bufs=2))
            pexp = ctx.enter_context(tc.tile_pool(name="pexp", bufs=24))
            pnorm = ctx.enter_context(tc.tile_pool(name="pnorm", bufs=2))
            pb2 = ctx.enter_context(tc.tile_pool(name="pb2", bufs=2))
            pstage = ctx.enter_context(tc.tile_pool(name="pstage", bufs=2))
            ps_score = ctx.enter_context(
                tc.tile_pool(name="ps_score", bufs=2, space="PSUM")
            )
            ps_pv = ctx.enter_context(tc.tile_pool(name="ps_pv", bufs=2, space="PSUM"))
            ps_m = ctx.enter_context(tc.tile_pool(name="ps_m", bufs=2, space="PSUM"))

            # ---- constants / weights (once) ----
            ident_h = pw.tile([128, 128], fp16)
            make_identity(nc, ident_h)
            wall_sb = pw.tile([128, 8, 128], fp16)
            nc.scalar.dma_start(out=wall_sb, in_=wall_d[:, :, :])
            w_sb = {n: wall_sb[:, i, :] for i, n in enumerate(WNAMES)}
            wv_sb = pw.tile([128, 128], fp16)
            nc.scalar.dma_start(out=wv_sb, in_=wv_d[:, :])
            sdr_sb = pw.tile([64, 256], f8)
            nc.scalar.dma_start(out=sdr_sb, in_=sdr_d[:, :])
            sdr_ap = sdr_sb.rearrange("p (two m) -> p two m", two=2)
            b4_sb = pw.tile([4, 128], fp16)
            nc.scalar.dma_start(out=b4_sb, in_=b4_d[:, :])
            b_sb = pw.tile([128, 1], f32)
            nc.scalar.dma_start(out=b_sb, in_=b_d[:, :])
            # v_aug: per n-chunk, 8 heads at 32-col blocks: cols 32h..32h+15 =
            # v head h, col 32h+16 = 1.0 (denominator row), rest zero.
            # Two buffers: batch b+1's V projection lands while batch b's PV
            # still reads its own.
            v_augs = []
            for vi in range(3):
                va = pw.tile([128, 4, 256], bf16, name=f"v_aug{vi}", tag=f"v_aug{vi}")
                nc.vector.memset(va, 0.0)
                va_blk = va.rearrange("p c (h x) -> p c h x", x=32)
                nc.vector.memset(va_blk[:, :, :, 16:17], 1.0)
                v_augs.append(va)

            def emit_loads(b):
                x = {}
                x["nodes"] = pin.tile([128, 4, 128], fp16, name="x_nodes", tag="x_nodes")
                nc.sync.dma_start(
                    out=x["nodes"], in_=nodes_d[b].rearrange("(c p) e -> p c e", p=128)
                )
                x["q1"] = pin.tile([128, 4, 128], fp16, name="x_q1", tag="x_q1")
                nc.sync.dma_start(
                    out=x["q1"], in_=q1_d[b].rearrange("(c p) e -> p c e", p=128)
                )
                x["last"] = pin.tile([128, 4, 128], fp16, name="x_last", tag="x_last")
                nc.sync.dma_start(
                    out=x["last"], in_=last_d[b].rearrange("(c p) e -> p c e", p=128)
                )
                x["mdr"] = pmask.tile([64, 4, 2, 512], f8, name="mdr", tag="mdr")
                nc.sync.dma_start(out=x["mdr"], in_=mdr_d[b])
                x["m16"] = pmask.tile([128, 4, 512], fp16, name="m16", tag="m16")
                nc.sync.dma_start(
                    out=x["m16"], in_=m16_d[b].rearrange("(c p) n -> p c n", p=128)
                )
                return x

            def emit_btp(b, x):
                """Transposes + projections for a batch (PE-early stage)."""
                st = {"v_aug": v_augs[b % 3]}

                def transpose_to(dst_name, src_nat, eng):
                    tp = ps_m.tile([128, 512], fp16, tag="m")
                    for c in range(4):
                        nc.tensor.transpose(
                            tp[:, c * 128 : (c + 1) * 128], src_nat[:, c, :], ident_h
                        )
                    dst = pxt.tile([128, 512], fp16, tag=dst_name)
                    nc.vector.tensor_copy(dst, tp)
                    return dst

                nodesT = transpose_to("nodesT", x["nodes"], nc.gpsimd)
                q1T = transpose_to("q1T", x["q1"], nc.gpsimd)
                lastT = transpose_to("lastT", x["last"], nc.gpsimd)
                st["nodesT"] = nodesT

                qT = []
                for g4 in range(2):  # head groups 0-3 / 4-7
                    ps = ps_m.tile([128, 512], f32, tag="m")
                    nc.tensor.matmul(ps, w_sb[f"Wq1p{g4}"], q1T, start=True, stop=False)
                    nc.tensor.matmul(
                        ps, w_sb[f"Wqlp{g4}"], lastT, start=False, stop=True
                    )
                    t = pproj.tile([128, 512], f32r, tag=f"qT{g4}")
                    nc.vector.tensor_copy(t, ps)
                    qT.append(t)
                kT = []
                for g4 in range(2):
                    ps = ps_m.tile([128, 512], f32, tag="m")
                    nc.tensor.matmul(ps, w_sb[f"Wkp{g4}"], nodesT, start=True, stop=True)
                    t = pproj.tile([128, 512], f32r, tag=f"kT{g4}")
                    nc.vector.tensor_copy(t, ps)
                    kT.append(t)
                st["qT"], st["kT"] = qT, kT
                # v natural [n, hd] into one psum tile, scattered to v_aug blocks
                vps = ps_m.tile([128, 512], f32, tag="m")
                for c in range(4):
                    nc.tensor.matmul(
                        vps[:, c * 128 : (c + 1) * 128],
                        nodesT[:, c * 128 : (c + 1) * 128],
                        wv_sb,
                        start=True,
                        stop=True,
                    )
                v_aug_f = st["v_aug"].rearrange("p c (h i d) -> p c h i d", i=2, d=16)
                nc.vector.tensor_copy(
                    v_aug_f[:, :, :, 0, :],
                    vps.rearrange("p (c h d) -> p c h d", c=4, d=16),
                )
                return st

            def emit_qke(x, st):
                """QK + fp8-DR mask + exp for all 16 tiles (both halves)."""
                qT, kT, mdr = st["qT"], st["kT"], x["mdr"]
                expps = {}
                for half in range(2):
                    for hp in range(2):
                        j0, j1 = 2 * hp, 2 * hp + 1
                        for c in range(4):
                            sc = ps_score.tile([128, 1024], f32, tag="sc")
                            for idx, j in enumerate((j0, j1)):
                                nc.tensor.matmul(
                                    sc[:, idx * 512 : (idx + 1) * 512],
                                    kT[half][
                                        32 * j : 32 * j + 16,
                                        c * 128 : (c + 1) * 128,
                                    ],
                                    qT[half][32 * j : 32 * j + 16, :],
                                    start=True,
                                    stop=False,
                                    tile_position=(32 * j, 0),
                                )
                            # += maskT via fp8 DoubleRow identity-expansion
                            for idx in range(2):
                                nc.tensor.matmul(
                                    sc[:, idx * 512 : (idx + 1) * 512],
                                    sdr_ap,
                                    mdr[:, c],
                                    start=False,
                                    stop=True,
                                    perf_mode=DR,
                                )
                            expp = pexp.tile([128, 1024], bf16, tag="expp")
                            nc.scalar.activation(expp, sc, AF.Exp)
                            expps[(half, hp, c)] = expp
                return expps

            def emit_pv_norm(st, expps):
                """PV accumulation + per-half normalization."""
                v_aug = st["v_aug"]
                u_norm = []
                for half in range(2):
                    pv = ps_pv.tile([128, 512], f32, tag="pv")
                    for hp in range(2):
                        j0, j1 = 2 * hp, 2 * hp + 1
                        for idx, j in enumerate((j0, j1)):
                            vcol = 32 * (half * 4 + j)  # head's v_aug block
                            for c in range(4):
                                nc.tensor.matmul(
                                    pv[32 * j : 32 * j + 32, :],
                                    v_aug[:, c, vcol : vcol + 32],
                                    expps[(half, hp, c)][:, idx * 512 : (idx + 1) * 512],
                                    start=(c == 0),
                                    stop=(c == 3),
                                    tile_position=(0, 32 * j),
                                )
                    u_sb = pnorm.tile([128, 512], fp16, tag=f"u_sb{half}")
                    nc.vector.tensor_copy(u_sb, pv)
                    # den rows sit at partitions 32k+16; lane-crossing goes
                    # through PE transposes (strided FREE reads are legal,
                    # strided partition reads are not).
                    uT = ps_m.tile([128, 512], fp16, tag="m")
                    for c in range(4):
                        nc.tensor.transpose(
                            uT[:, c * 128 : (c + 1) * 128],
                            u_sb[:, c * 128 : (c + 1) * 128],
                            ident_h,
                        )
                    uTv = uT.rearrange("p (c k x) -> p c k x", c=4, x=32)
                    recn = pnorm.tile([128, 4, 4], fp16, tag="recn")
                    with nc.allow_low_precision(reason="1/den fits fp16"):
                        nc.vector.reciprocal(recn, uTv[:, :, :, 16])
                    recT = ps_m.tile([4, 512], fp16, tag="m")
                    for c in range(4):
                        nc.tensor.transpose(
                            recT[:, c * 128 : (c + 1) * 128], recn[:, c, :], ident_h
                        )
                    recT_sb = pnorm.tile([4, 512], fp16, tag="recT_sb")
                    nc.vector.tensor_copy(recT_sb, recT)
                    bc_ps = ps_m.tile([128, 512], f32, tag="m")
                    nc.tensor.matmul(bc_ps, b4_sb, recT_sb, start=True, stop=True)
                    un = pnorm.tile([128, 512], fp16, tag=f"un{half}")
                    nc.vector.tensor_mul(un, u_sb, bc_ps)
                    u_norm.append(un)
                return u_norm

            def emit_d_fine(b, x, st, u_norm):
                """Last-batch variant: per-gc chains so the tail pipelines."""
                mh_ps = ps_m.tile([128, 512], f32, tag="m")
                nc.tensor.matmul(mh_ps, w_sb["Wcp0"], u_norm[0], start=True, stop=False)
                nc.tensor.matmul(mh_ps, w_sb["Wcp1"], u_norm[1], start=False, stop=True)
                mhT = pnorm.tile([128, 512], fp16, tag="mhT")
                nc.vector.tensor_scalar_add(mhT, mh_ps, b_sb[:, 0:1])

                nodesT = st["nodesT"]
                tanh_sb = pb2.tile([128, 4, 512], fp16, tag="tanh")
                z = pb2.tile([128, 4, 512], fp16, tag="z")
                p_sb = pb2.tile([128, 4, 512], fp16, tag="p")
                den = pnorm.tile([128, 4], f32, tag="den")
                recipden = pnorm.tile([128, 4], f32, tag="recipden")
                stage = pstage.tile([128, 4, 512], bf16, tag="stage")
                out_ap = probs_d[b].rearrange("(c p) n -> p c n", p=128)
                for gc in range(4):
                    s2 = ps_m.tile([128, 512], f32, tag="m")
                    nc.tensor.matmul(
                        s2,
                        mhT[:, gc * 128 : (gc + 1) * 128],
                        nodesT,
                        start=True,
                        stop=True,
                    )
                    sl = slice(gc, gc + 1)
                    nc.scalar.activation(
                        tanh_sb[:, gc, :], s2, AF.Tanh, scale=1.0 / SQRT_E
                    )
                    nc.vector.tensor_add(z[:, sl, :], tanh_sb[:, sl, :], x["m16"][:, sl, :])
                    nc.scalar.activation(p_sb[:, gc, :], z[:, gc, :], AF.Exp, scale=10.0)
                    nc.vector.tensor_reduce(
                        den[:, gc : gc + 1],
                        p_sb[:, gc, :],
                        mybir.AxisListType.X,
                        mybir.AluOpType.add,
                    )
                    nc.vector.reciprocal(recipden[:, gc : gc + 1], den[:, gc : gc + 1])
                    nc.vector.tensor_scalar_mul(
                        stage[:, gc, :], p_sb[:, gc, :], recipden[:, gc : gc + 1]
                    )
                    nc.gpsimd.dma_start(out=out_ap[:, sl, :], in_=stage[:, sl, :])

            def emit_d(b, x, st, u_norm):
                """Combine + branch2 + output."""
                mh_ps = ps_m.tile([128, 512], f32, tag="m")
                nc.tensor.matmul(mh_ps, w_sb["Wcp0"], u_norm[0], start=True, stop=False)
                nc.tensor.matmul(mh_ps, w_sb["Wcp1"], u_norm[1], start=False, stop=True)
                mhT = pnorm.tile([128, 512], fp16, tag="mhT")
                nc.vector.tensor_scalar_add(mhT, mh_ps, b_sb[:, 0:1])

                nodesT = st["nodesT"]
                tanh_sb = pb2.tile([128, 4, 512], fp16, tag="tanh")
                z = pb2.tile([128, 4, 512], fp16, tag="z")
                p_sb = pb2.tile([128, 4, 512], fp16, tag="p")
                den = pnorm.tile([128, 4], f32, tag="den")
                recipden = pnorm.tile([128, 4], f32, tag="recipden")
                for gp in range(2):  # gc pairs share one 2-bank psum tile
                    s2 = ps_score.tile([128, 1024], f32, tag="sc")
                    for gi in range(2):
                        gc = 2 * gp + gi
                        nc.tensor.matmul(
                            s2[:, gi * 512 : (gi + 1) * 512],
                            mhT[:, gc * 128 : (gc + 1) * 128],
                            nodesT,
                            start=True,
                            stop=True,
                        )
                    sl = slice(2 * gp, 2 * gp + 2)
                    nc.scalar.activation(
                        tanh_sb[:, sl, :],
                        s2.rearrange("p (gi n) -> p gi n", gi=2),
                        AF.Tanh,
                        scale=1.0 / SQRT_E,
                    )
                    # z = tanh + mask/10 (fp16, DVE 2x); exp applies the x10
                    nc.vector.tensor_add(z[:, sl, :], tanh_sb[:, sl, :], x["m16"][:, sl, :])
                    nc.scalar.activation(p_sb[:, sl, :], z[:, sl, :], AF.Exp, scale=10.0)
                    for gi in range(2):
                        gc = 2 * gp + gi
                        nc.vector.tensor_reduce(
                            den[:, gc : gc + 1],
                            p_sb[:, gc, :],
                            mybir.AxisListType.X,
                            mybir.AluOpType.add,
                        )
                nc.vector.reciprocal(recipden, den)
                stage = pstage.tile([128, 4, 512], bf16, tag="stage")
                out_ap = probs_d[b].rearrange("(c p) n -> p c n", p=128)
                for gp in range(2):  # interleave scale + half-DMA
                    for gi in range(2):
                        gc = 2 * gp + gi
                        nc.vector.tensor_scalar_mul(
                            stage[:, gc, :], p_sb[:, gc, :], recipden[:, gc : gc + 1]
                        )
                    nc.gpsimd.dma_start(
                        out=out_ap[:, 2 * gp : 2 * gp + 2, :],
                        in_=stage[:, 2 * gp : 2 * gp + 2, :],
                    )

            # ---- software-pipelined batch loop (skewed: next batch's
            # QK/exp units are emitted before this batch's tail so ACT never
            # starves during the combine/branch2 dependency chain) ----
            xs = [None] * BL
            sts = [None] * BL
            exps = [None] * BL
            xs[0] = emit_loads(0)
            sts[0] = emit_btp(0, xs[0])
            if BL > 1:
                xs[1] = emit_loads(1)
                sts[1] = emit_btp(1, xs[1])
            exps[0] = emit_qke(xs[0], sts[0])
            for b in range(BL):
                if b + 2 < BL:
                    xs[b + 2] = emit_loads(b + 2)
                    sts[b + 2] = emit_btp(b + 2, xs[b + 2])
                if b + 1 < BL:
                    exps[b + 1] = emit_qke(xs[b + 1], sts[b + 1])
                u_norm = emit_pv_norm(sts[b], exps[b])
                if b == BL - 1:
                    emit_d_fine(b, xs[b], sts[b], u_norm)
                else:
                    emit_d(b, xs[b], sts[b], u_norm)
                xs[b], exps[b] = None, None

    if legalize:
        _legalize_waits(nc)
    return nc


def _prep_weights(inputs):
    def pad4(W):
        Wp0 = np.zeros((E, 128), np.float32)
        Wp1 = np.zeros((E, 128), np.float32)
        for j in range(4):
            Wp0[:, 32 * j : 32 * j + 16] = W[:, 16 * j : 16 * j + 16]
            Wp1[:, 32 * j : 32 * j + 16] = W[:, 64 + 16 * j : 64 + 16 * j + 16]
        return Wp0, Wp1

    import ml_dtypes

    s = np.float32(1.0 / np.sqrt(np.float32(D)))  # 0.25 folded into q weights
    Wq1p0, Wq1p1 = pad4(np.asarray(inputs["Wq_first"], np.float32) * s)
    Wqlp0, Wqlp1 = pad4(np.asarray(inputs["Wq_last"], np.float32) * s)
    Wkp0, Wkp1 = pad4(np.asarray(inputs["Wk"], np.float32))
    # W_comb rows padded to the 32-blocks of u_norm: row 32j+d of half g4 is
    # W_comb[(4*g4+j)*16 + d]
    Wc = np.asarray(inputs["W_comb"], np.float32)
    Wcp = np.zeros((2, 128, E), np.float32)
    for g4 in range(2):
        for j in range(4):
            Wcp[g4, 32 * j : 32 * j + 16, :] = Wc[(4 * g4 + j) * 16 : (4 * g4 + j) * 16 + 16, :]
    # DoubleRow expansion stationary: S[k, i, m] = (m == 2k+i)
    Sdr = np.zeros((64, 2, 128), np.float32)
    k_idx = np.arange(64)
    Sdr[k_idx, 0, 2 * k_idx] = 1.0
    Sdr[k_idx, 1, 2 * k_idx + 1] = 1.0
    # B4[k, m] = 1 for m in [32k, 32k+16)
    B4 = np.zeros((4, 128), np.float32)
    for k in range(4):
        B4[k, 32 * k : 32 * k + 16] = 1.0
    # order must match WNAMES in _build_nc
    wall = np.stack(
        [Wq1p0, Wq1p1, Wqlp0, Wqlp1, Wkp0, Wkp1, Wcp[0], Wcp[1]], axis=1
    )  # [E, 8, 128]
    return {
        "wall": np.ascontiguousarray(wall.astype(np.float16)),
        "Wv": np.asarray(inputs["Wv"], np.float16),
        "Sdr": Sdr.reshape(64, 256).astype(ml_dtypes.float8_e4m3fn),
        "B4": B4.astype(np.float16),
        "bc": np.asarray(inputs["b_comb"], np.float32).reshape(E, 1),
    }


def _prep_masks(mask):
    """mask: [B, G, N] f32 (0 / -1e9).  Returns fp8 pair-packed transposed
    mask (0/-240) and fp16 natural mask/10 (0/-24)."""
    import ml_dtypes

    neg = mask < 0
    # mdr[b, k, c, i, g] = M240[b, g, n=128c+2k+i]
    negT = neg.transpose(0, 2, 1)  # [B, N, G]
    negT = negT.reshape(mask.shape[0], 4, 64, 2, G).transpose(0, 2, 1, 3, 4)
    mdr = np.where(negT, np.float32(-240.0), np.float32(0.0)).astype(
        ml_dtypes.float8_e4m3fn
    )
    m16 = np.where(neg, np.float16(-24.0), np.float16(0.0))
    return np.ascontiguousarray(mdr), np.ascontiguousarray(m16)


def _make_in_maps(inputs):
    w = _prep_weights(inputs)
    nodes = np.ascontiguousarray(np.asarray(inputs["encoded_nodes"], np.float16))
    q1 = np.ascontiguousarray(np.asarray(inputs["encoded_q1"], np.float16))
    last = np.ascontiguousarray(np.asarray(inputs["encoded_last_node"], np.float16))
    mdr, m16 = _prep_masks(np.asarray(inputs["ninf_mask"], np.float32))

    in_maps = []
    for i in range(NCORES):
        sl = slice(i * BL, (i + 1) * BL)
        in_maps.append(
            {
                "nodes": nodes[sl],
                "q1": q1[sl],
                "last": last[sl],
                "mdr": mdr[sl],
                "m16": m16[sl],
                **w,
            }
        )
    return in_maps


def run(inputs, trace=False):
    from concourse.bass_utils import run_bass_kernel_spmd

    if "nc" not in _CACHE:
        _CACHE["nc"] = _build_nc()
    nc = _CACHE["nc"]

    in_maps = _make_in_maps(inputs)
    try:
        res = run_bass_kernel_spmd(nc, in_maps, list(range(NCORES)), trace=trace)
    except Exception:
        # The first execution of a freshly compiled NEFF occasionally dies
        # with NRT_EXEC_UNIT_UNRECOVERABLE on this stack; a retry with the
        # cached NEFF has always succeeded.
        res = run_bass_kernel_spmd(nc, in_maps, list(range(NCORES)), trace=trace)
    out = np.concatenate([res.results[i]["probs"] for i in range(NCORES)], axis=0)
    return out, res


def kernel(**inputs) -> np.ndarray:
    out, _ = run(inputs, trace=False)
    return out


# revision 42
# speedup vs baseline: 1.6180x; 1.0115x over previous
"""Trainium2 Bass kernel for nn_Decoder_14680198217759.

Multi-head attention decoder (B=32, G=N=512, E=128, H=8, D=16), pure data
parallel over 8 NeuronCores (4 batches/core).

Layout strategy per batch (all on one core):
  - Activations transposed via PE (bf16 identity => 1 cycle/row, exact
    permutation path) so E sits on partitions: xT [E, G] in f32r.
  - Q/K projections produce qT/kT head-padded (head j of a 4-head group at
    partition offset 32j) so per-head K=16 score matmuls use legal base
    partitions via tile_position.
  - Scores TRANSPOSED: scoreT_h [n, g], head pair packed in one [128,1024]
    PSUM tile.  The rank-3 mask rides the PE as an fp8 DoubleRow matmul:
    stationary S[k,i,m] = (m==2k+i) expands a pair-packed fp8 mask (0/-240)
    moving operand at 0.5 cycles/row -- exact, half the cost of the bf16
    identity trick.  exp(s-240) underflows to exactly 0 in fp32, matching
    the reference's -1e9 mask.
  - exp on ACT (PSUM -> SBUF, bf16).  PV contracts exp with v_aug (v plus a
    ones column per 32-block) giving out^T plus softmax denominators.
  - Normalization stays transposed: reciprocal of the strided den rows, a
    [4,128] 0/1 broadcast matmul expands 1/den across each head's 16 rows,
    one DVE multiply produces u_norm; combine uses padded W_comb halves.
  - Branch 2 (single-head scoring) natural [g, n]: tanh (scale 1/sqrt(E))
    to fp16, fp16 mask/10 added on DVE (2x mode), exp with scale=10 and
    accum_out denominators, 4x-mode tensor_scalar scale to bf16, casting
    DMA (gpsimd) upconverts to the f32 output.
"""

import numpy as np

B, G, N, E, H, D = 32, 512, 512, 128, 8, 16
SQRT_E = 11.313708498984761
NCORES = 8
BL = B // NCORES  # batches per core

_CACHE = {}


# --------------------------------------------------------------------------
# BIR wait legalization: this toolchain's walrus accepts at most ONE sem wait
# per instruction; Tile's scheduler can emit more (notably on the kernel-tail
# drain). Split excess waits onto same-engine NoOps placed directly before
# the offending instruction (same-queue program order keeps the semantics).
# --------------------------------------------------------------------------
def _legalize_waits(nc, max_waits=1):
    import concourse.mybir as mybir

    n_split = 0
    for f in nc.m.functions:
        for bb in f.blocks:
            out = []
            for ins in bb.instructions:
                si = ins.sync_info
                waits = list(si.on_wait) if si and si.on_wait else []
                if len(waits) > max_waits:
                    while len(waits) > max_waits:
                        chunk, waits = waits[:max_waits], waits[max_waits:]
                        nop = mybir.InstNoOp(
                            name=f"I-waitfix-{nc.next_id()}", ins=[], outs=[]
                        )
                        nop.engine = ins.engine
                        nop.sync_info = mybir.SyncInfo(on_wait=chunk, on_update=[])
                        out.append(nop)
                        n_split += 1
                    ins.sync_info = mybir.SyncInfo(
                        on_wait=waits, on_update=list(si.on_update or [])
                    )
                out.append(ins)
            bb.instructions[:] = out
    return n_split


def _build_nc(legalize=True):
    import concourse.bass as bass
    import concourse.mybir as mybir
    import concourse.tile as tile
    from concourse.masks import make_identity

    f32 = mybir.dt.float32
    f32r = mybir.dt.float32r
    bf16 = mybir.dt.bfloat16
    fp16 = mybir.dt.float16
    f8 = mybir.dt.float8e4
    AF = mybir.ActivationFunctionType
    DR = mybir.MatmulPerfMode.DoubleRow

    nc = bass.Bass()

    nodes_d = nc.dram_tensor("nodes", [BL, N, E], fp16, kind="ExternalInput")
    q1_d = nc.dram_tensor("q1", [BL, G, E], fp16, kind="ExternalInput")
    last_d = nc.dram_tensor("last", [BL, G, E], fp16, kind="ExternalInput")
    # pair-packed transposed fp8 mask: mdr[b, k, c, i, g] = M240[b, g, 128c+2k+i]
    mdr_d = nc.dram_tensor("mdr", [BL, 64, 4, 2, G], f8, kind="ExternalInput")
    # natural fp16 mask / 10 (branch2 exp runs with scale=10)
    m16_d = nc.dram_tensor("m16", [BL, G, N], fp16, kind="ExternalInput")
    # 8 padded f32r weights combined into one DMA: [E, i, 128]
    WNAMES = ["Wq1p0", "Wq1p1", "Wqlp0", "Wqlp1", "Wkp0", "Wkp1", "Wcp0", "Wcp1"]
    wall_d = nc.dram_tensor("wall", [E, 8, 128], fp16, kind="ExternalInput")
    wv_d = nc.dram_tensor("Wv", [E, 128], fp16, kind="ExternalInput")
    sdr_d = nc.dram_tensor("Sdr", [64, 256], f8, kind="ExternalInput")
    b4_d = nc.dram_tensor("B4", [4, 128], fp16, kind="ExternalInput")
    b_d = nc.dram_tensor("bc", [E, 1], f32, kind="ExternalInput")
    probs_d = nc.dram_tensor("probs", [BL, G, N], f32, kind="ExternalOutput")

    with tile.TileContext(nc) as tc:
        import contextlib

        with contextlib.ExitStack() as ctx:
            pw = ctx.enter_context(tc.tile_pool(name="pw", bufs=1))
            pin = ctx.enter_context(tc.tile_pool(name="pin", bufs=3))
            pmask = ctx.enter_context(tc.tile_pool(name="pmask", bufs=3))
            pxt = ctx.enter_context(tc.tile_pool(name="pxt", bufs=3))
            pproj = ctx.enter_context(tc.tile_pool(name="pproj", bufs=2))
            pexp = ctx.enter_context(tc.tile_pool(name="pexp", bufs=28))
            pnorm = ctx.enter_context(tc.tile_pool(name="pnorm", bufs=2))
            pb2 = ctx.enter_context(tc.tile_pool(name="pb2", bufs=2))
            pstage = ctx.enter_context(tc.tile_pool(name="pstage", bufs=2))
            ps_score = ctx.enter_context(
                tc.tile_pool(name="ps_score", bufs=2, space="PSUM")
            )
            ps_pv = ctx.enter_context(tc.tile_pool(name="ps_pv", bufs=2, space="PSUM"))
            ps_m = ctx.enter_context(tc.tile_pool(name="ps_m", bufs=2, space="PSUM"))

            # ---- constants / weights (once) ----
            ident_h = pw.tile([128, 128], fp16)
            make_identity(nc, ident_h)
            wall_sb = pw.tile([128, 8, 128], fp16)
            nc.scalar.dma_start(out=wall_sb, in_=wall_d[:, :, :])
            w_sb = {n: wall_sb[:, i, :] for i, n in enumerate(WNAMES)}
            wv_sb = pw.tile([128, 128], fp16)
            nc.scalar.dma_start(out=wv_sb, in_=wv_d[:, :])
            sdr_sb = pw.tile([64, 256], f8)
            nc.scalar.dma_start(out=sdr_sb, in_=sdr_d[:, :])
            sdr_ap = sdr_sb.rearrange("p (two m) -> p two m", two=2)
            b4_sb = pw.tile([4, 128], fp16)
            nc.scalar.dma_start(out=b4_sb, in_=b4_d[:, :])
            b_sb = pw.tile([128, 1], f32)
            nc.scalar.dma_start(out=b_sb, in_=b_d[:, :])
            # v_aug: per n-chunk, 8 heads at 32-col blocks: cols 32h..32h+15 =
            # v head h, col 32h+16 = 1.0 (denominator row), rest zero.
            # Two buffers: batch b+1's V projection lands while batch b's PV
            # still reads its own.
            v_augs = []
            for vi in range(3):
                va = pw.tile([128, 4, 256], bf16, name=f"v_aug{vi}", tag=f"v_aug{vi}")
                nc.vector.memset(va, 0.0)
                va_blk = va.rearrange("p c (h x) -> p c h x", x=32)
                nc.vector.memset(va_blk[:, :, :, 16:17], 1.0)
                v_augs.append(va)

            def emit_loads(b):
                x = {}
                x["nodes"] = pin.tile([128, 4, 128], fp16, name="x_nodes", tag="x_nodes")
                nc.sync.dma_start(
                    out=x["nodes"], in_=nodes_d[b].rearrange("(c p) e -> p c e", p=128)
                )
                x["q1"] = pin.tile([128, 4, 128], fp16, name="x_q1", tag="x_q1")
                nc.sync.dma_start(
                    out=x["q1"], in_=q1_d[b].rearrange("(c p) e -> p c e", p=128)
                )
                x["last"] = pin.tile([128, 4, 128], fp16, name="x_last", tag="x_last")
                nc.sync.dma_start(
                    out=x["last"], in_=last_d[b].rearrange("(c p) e -> p c e", p=128)
                )
                x["mdr"] = pmask.tile([64, 4, 2, 512], f8, name="mdr", tag="mdr")
                nc.sync.dma_start(out=x["mdr"], in_=mdr_d[b])
                x["m16"] = pmask.tile([128, 4, 512], fp16, name="m16", tag="m16")
                nc.sync.dma_start(
                    out=x["m16"], in_=m16_d[b].rearrange("(c p) n -> p c n", p=128)
                )
                return x

            def emit_btp(b, x):
                """Transposes + projections for a batch (PE-early stage)."""
                st = {"v_aug": v_augs[b % 3]}

                def transpose_to(dst_name, src_nat, eng):
                    tp = ps_m.tile([128, 512], fp16, tag="m")
                    for c in range(4):
                        nc.tensor.transpose(
                            tp[:, c * 128 : (c + 1) * 128], src_nat[:, c, :], ident_h
                        )
                    dst = pxt.tile([128, 512], fp16, tag=dst_name)
                    nc.vector.tensor_copy(dst, tp)
                    return dst

                nodesT = transpose_to("nodesT", x["nodes"], nc.gpsimd)
                q1T = transpose_to("q1T", x["q1"], nc.gpsimd)
                lastT = transpose_to("lastT", x["last"], nc.gpsimd)
                st["nodesT"] = nodesT

                qT = []
                for g4 in range(2):  # head groups 0-3 / 4-7
                    ps = ps_m.tile([128, 512], f32, tag="m")
                    nc.tensor.matmul(ps, w_sb[f"Wq1p{g4}"], q1T, start=True, stop=False)
                    nc.tensor.matmul(
                        ps, w_sb[f"Wqlp{g4}"], lastT, start=False, stop=True
                    )
                    t = pproj.tile([128, 512], f32r, tag=f"qT{g4}")
                    nc.vector.tensor_copy(t, ps)
                    qT.append(t)
                kT = []
                for g4 in range(2):
                    ps = ps_m.tile([128, 512], f32, tag="m")
                    nc.tensor.matmul(ps, w_sb[f"Wkp{g4}"], nodesT, start=True, stop=True)
                    t = pproj.tile([128, 512], f32r, tag=f"kT{g4}")
                    nc.vector.tensor_copy(t, ps)
                    kT.append(t)
                st["qT"], st["kT"] = qT, kT
                # v natural [n, hd] into one psum tile, scattered to v_aug blocks
                vps = ps_m.tile([128, 512], f32, tag="m")
                for c in range(4):
                    nc.tensor.matmul(
                        vps[:, c * 128 : (c + 1) * 128],
                        nodesT[:, c * 128 : (c + 1) * 128],
                        wv_sb,
                        start=True,
                        stop=True,
                    )
                v_aug_f = st["v_aug"].rearrange("p c (h i d) -> p c h i d", i=2, d=16)
                nc.vector.tensor_copy(
                    v_aug_f[:, :, :, 0, :],
                    vps.rearrange("p (c h d) -> p c h d", c=4, d=16),
                )
                return st

            def emit_qke(x, st):
                """QK + fp8-DR mask + exp for all 16 tiles (both halves)."""
                qT, kT, mdr = st["qT"], st["kT"], x["mdr"]
                expps = {}
                for half in range(2):
                    for hp in range(2):
                        j0, j1 = 2 * hp, 2 * hp + 1
                        for c in range(4):
                            sc = ps_score.tile([128, 1024], f32, tag="sc")
                            for idx, j in enumerate((j0, j1)):
                                nc.tensor.matmul(
                                    sc[:, idx * 512 : (idx + 1) * 512],
                                    kT[half][
                                        32 * j : 32 * j + 16,
                                        c * 128 : (c + 1) * 128,
                                    ],
                                    qT[half][32 * j : 32 * j + 16, :],
                                    start=True,
                                    stop=False,
                                    tile_position=(32 * j, 0),
                                )
                            # += maskT via fp8 DoubleRow identity-expansion
                            for idx in range(2):
                                nc.tensor.matmul(
                                    sc[:, idx * 512 : (idx + 1) * 512],
                                    sdr_ap,
                                    mdr[:, c],
                                    start=False,
                                    stop=True,
                                    perf_mode=DR,
                                )
                            expp = pexp.tile([128, 1024], bf16, tag="expp")
                            nc.scalar.activation(expp, sc, AF.Exp)
                            expps[(half, hp, c)] = expp
                return expps

            def emit_pv_norm(st, expps, last=False):
                """PV accumulation + per-half normalization.  For the final
                batch the PSUM->SBUF copies ride the otherwise-idle ACT
                engine to shorten the tail's serial DVE chain."""
                v_aug = st["v_aug"]
                u_norm = []
                for half in range(2):
                    pv = ps_pv.tile([128, 512], f32, tag="pv")
                    for hp in range(2):
                        j0, j1 = 2 * hp, 2 * hp + 1
                        for idx, j in enumerate((j0, j1)):
                            vcol = 32 * (half * 4 + j)  # head's v_aug block
                            for c in range(4):
                                nc.tensor.matmul(
                                    pv[32 * j : 32 * j + 32, :],
                                    v_aug[:, c, vcol : vcol + 32],
                                    expps[(half, hp, c)][:, idx * 512 : (idx + 1) * 512],
                                    start=(c == 0),
                                    stop=(c == 3),
                                    tile_position=(0, 32 * j),
                                )
                    u_sb = pnorm.tile([128, 512], fp16, tag=f"u_sb{half}")
                    if last:
                        nc.scalar.copy(u_sb, pv)
                    else:
                        nc.vector.tensor_copy(u_sb, pv)
                    # den rows sit at partitions 32k+16; lane-crossing goes
                    # through PE transposes (strided FREE reads are legal,
                    # strided partition reads are not).
                    uT = ps_m.tile([128, 512], fp16, tag="m")
                    for c in range(4):
                        nc.tensor.transpose(
                            uT[:, c * 128 : (c + 1) * 128],
                            u_sb[:, c * 128 : (c + 1) * 128],
                            ident_h,
                        )
                    uTv = uT.rearrange("p (c k x) -> p c k x", c=4, x=32)
                    recn = pnorm.tile([128, 4, 4], fp16, tag="recn")
                    with nc.allow_low_precision(reason="1/den fits fp16"):
                        nc.vector.reciprocal(recn, uTv[:, :, :, 16])
                    recT = ps_m.tile([4, 512], fp16, tag="m")
                    for c in range(4):
                        nc.tensor.transpose(
                            recT[:, c * 128 : (c + 1) * 128], recn[:, c, :], ident_h
                        )
                    recT_sb = pnorm.tile([4, 512], fp16, tag="recT_sb")
                    if last:
                        nc.scalar.copy(recT_sb, recT)
                    else:
                        nc.vector.tensor_copy(recT_sb, recT)
                    bc_ps = ps_m.tile([128, 512], f32, tag="m")
                    nc.tensor.matmul(bc_ps, b4_sb, recT_sb, start=True, stop=True)
                    un = pnorm.tile([128, 512], fp16, tag=f"un{half}")
                    nc.vector.tensor_mul(un, u_sb, bc_ps)
                    u_norm.append(un)
                return u_norm

            def emit_d_fine(b, x, st, u_norm):
                """Last-batch variant: per-gc chains so the tail pipelines."""
                mh_ps = ps_m.tile([128, 512], f32, tag="m")
                nc.tensor.matmul(mh_ps, w_sb["Wcp0"], u_norm[0], start=True, stop=False)
                nc.tensor.matmul(mh_ps, w_sb["Wcp1"], u_norm[1], start=False, stop=True)
                mhT = pnorm.tile([128, 512], fp16, tag="mhT")
                nc.scalar.activation(mhT, mh_ps, AF.Identity, bias=b_sb[:, 0:1])

                nodesT = st["nodesT"]
                tanh_sb = pb2.tile([128, 4, 512], fp16, tag="tanh")
                z = pb2.tile([128, 4, 512], fp16, tag="z")
                p_sb = pb2.tile([128, 4, 512], fp16, tag="p")
                den = pnorm.tile([128, 4], f32, tag="den")
                recipden = pnorm.tile([128, 4], f32, tag="recipden")
                stage = pstage.tile([128, 4, 512], bf16, tag="stage")
                out_ap = probs_d[b].rearrange("(c p) n -> p c n", p=128)
                for gc in range(4):
                    s2 = ps_m.tile([128, 512], f32, tag="m")
                    nc.tensor.matmul(
                        s2,
                        mhT[:, gc * 128 : (gc + 1) * 128],
                        nodesT,
                        start=True,
                        stop=True,
                    )
                    sl = slice(gc, gc + 1)
                    nc.scalar.activation(
                        tanh_sb[:, gc, :], s2, AF.Tanh, scale=1.0 / SQRT_E
                    )
                    nc.vector.tensor_add(z[:, sl, :], tanh_sb[:, sl, :], x["m16"][:, sl, :])
                    nc.scalar.activation(
                        p_sb[:, gc, :],
                        z[:, gc, :],
                        AF.Exp,
                        scale=10.0,
                        accum_out=den[:, gc : gc + 1],
                    )
                    nc.vector.reciprocal(recipden[:, gc : gc + 1], den[:, gc : gc + 1])
                    nc.vector.tensor_scalar_mul(
                        stage[:, gc, :], p_sb[:, gc, :], recipden[:, gc : gc + 1]
                    )
                    nc.gpsimd.dma_start(out=out_ap[:, sl, :], in_=stage[:, sl, :])

            def emit_d(b, x, st, u_norm):
                """Combine + branch2 + output."""
                mh_ps = ps_m.tile([128, 512], f32, tag="m")
                nc.tensor.matmul(mh_ps, w_sb["Wcp0"], u_norm[0], start=True, stop=False)
                nc.tensor.matmul(mh_ps, w_sb["Wcp1"], u_norm[1], start=False, stop=True)
                mhT = pnorm.tile([128, 512], fp16, tag="mhT")
                nc.vector.tensor_scalar_add(mhT, mh_ps, b_sb[:, 0:1])

                nodesT = st["nodesT"]
                tanh_sb = pb2.tile([128, 4, 512], fp16, tag="tanh")
                z = pb2.tile([128, 4, 512], fp16, tag="z")
                p_sb = pb2.tile([128, 4, 512], fp16, tag="p")
                den = pnorm.tile([128, 4], f32, tag="den")
                recipden = pnorm.tile([128, 4], f32, tag="recipden")
                for gp in range(2):  # gc pairs share one 2-bank psum tile
                    s2 = ps_score.tile([128, 1024], f32, tag="sc")
                    for gi in range(2):
                        gc = 2 * gp + gi
                        nc.tensor.matmul(
                            s2[:, gi * 512 : (gi + 1) * 512],
                            mhT[:, gc * 128 : (gc + 1) * 128],
                            nodesT,
                            start=True,
                            stop=True,
                        )
                    sl = slice(2 * gp, 2 * gp + 2)
                    nc.scalar.activation(
                        tanh_sb[:, sl, :],
                        s2.rearrange("p (gi n) -> p gi n", gi=2),
                        AF.Tanh,
                        scale=1.0 / SQRT_E,
                    )
                    # z = tanh + mask/10 (fp16, DVE 2x); exp applies the x10
                    nc.vector.tensor_add(z[:, sl, :], tanh_sb[:, sl, :], x["m16"][:, sl, :])
                    nc.scalar.activation(p_sb[:, sl, :], z[:, sl, :], AF.Exp, scale=10.0)
                    for gi in range(2):
                        gc = 2 * gp + gi
                        nc.vector.tensor_reduce(
                            den[:, gc : gc + 1],
                            p_sb[:, gc, :],
                            mybir.AxisListType.X,
                            mybir.AluOpType.add,
                        )
                nc.vector.reciprocal(recipden, den)
                stage = pstage.tile([128, 4, 512], bf16, tag="stage")
                out_ap = probs_d[b].rearrange("(c p) n -> p c n", p=128)
                for gp in range(2):  # interleave scale + half-DMA
                    for gi in range(2):
                        gc = 2 * gp + gi
                        nc.vector.tensor_scalar_mul(
                            stage[:, gc, :], p_sb[:, gc, :], recipden[:, gc : gc + 1]
                        )
                    nc.gpsimd.dma_start(
                        out=out_ap[:, 2 * gp : 2 * gp + 2, :],
                        in_=stage[:, 2 * gp : 2 * gp + 2, :],
                    )

            # ---- software-pipelined batch loop (skewed: next batch's
            # QK/exp units are emitted before this batch's tail so ACT never
            # starves during the combine/branch2 dependency chain) ----
            xs = [None] * BL
            sts = [None] * BL
            exps = [None] * BL
            xs[0] = emit_loads(0)
            sts[0] = emit_btp(0, xs[0])
            if BL > 1:
                xs[1] = emit_loads(1)
                sts[1] = emit_btp(1, xs[1])
            exps[0] = emit_qke(xs[0], sts[0])
            for b in range(BL):
                if b + 2 < BL:
                    xs[b + 2] = emit_loads(b + 2)
                    sts[b + 2] = emit_btp(b + 2, xs[b + 2])
                if b + 1 < BL:
                    exps[b + 1] = emit_qke(xs[b + 1], sts[b + 1])
                u_norm = emit_pv_norm(sts[b], exps[b], last=(b == BL - 1))
                if b == BL - 1:
                    emit_d_fine(b, xs[b], sts[b], u_norm)
                else:
                    emit_d(b, xs[b], sts[b], u_norm)
                xs[b], exps[b] = None, None

    if legalize:
        _legalize_waits(nc)
    return nc


def _prep_weights(inputs):
    def pad4(W):
        Wp0 = np.zeros((E, 128), np.float32)
        Wp1 = np.zeros((E, 128), np.float32)
        for j in range(4):
            Wp0[:, 32 * j : 32 * j + 16] = W[:, 16 * j : 16 * j + 16]
            Wp1[:, 32 * j : 32 * j + 16] = W[:, 64 + 16 * j : 64 + 16 * j + 16]
        return Wp0, Wp1

    import ml_dtypes

    s = np.float32(1.0 / np.sqrt(np.float32(D)))  # 0.25 folded into q weights
    Wq1p0, Wq1p1 = pad4(np.asarray(inputs["Wq_first"], np.float32) * s)
    Wqlp0, Wqlp1 = pad4(np.asarray(inputs["Wq_last"], np.float32) * s)
    Wkp0, Wkp1 = pad4(np.asarray(inputs["Wk"], np.float32))
    # W_comb rows padded to the 32-blocks of u_norm: row 32j+d of half g4 is
    # W_comb[(4*g4+j)*16 + d]
    Wc = np.asarray(inputs["W_comb"], np.float32)
    Wcp = np.zeros((2, 128, E), np.float32)
    for g4 in range(2):
        for j in range(4):
            Wcp[g4, 32 * j : 32 * j + 16, :] = Wc[(4 * g4 + j) * 16 : (4 * g4 + j) * 16 + 16, :]
    # DoubleRow expansion stationary: S[k, i, m] = (m == 2k+i)
    Sdr = np.zeros((64, 2, 128), np.float32)
    k_idx = np.arange(64)
    Sdr[k_idx, 0, 2 * k_idx] = 1.0
    Sdr[k_idx, 1, 2 * k_idx + 1] = 1.0
    # B4[k, m] = 1 for m in [32k, 32k+16)
    B4 = np.zeros((4, 128), np.float32)
    for k in range(4):
        B4[k, 32 * k : 32 * k + 16] = 1.0
    # order must match WNAMES in _build_nc
    wall = np.stack(
        [Wq1p0, Wq1p1, Wqlp0, Wqlp1, Wkp0, Wkp1, Wcp[0], Wcp[1]], axis=1
    )  # [E, 8, 128]
    return {
        "wall": np.ascontiguousarray(wall.astype(np.float16)),
        "Wv": np.asarray(inputs["Wv"], np.float16),
        "Sdr": Sdr.reshape(64, 256).astype(ml_dtypes.float8_e4m3fn),
        "B4": B4.astype(np.float16),
        "bc": np.asarray(inputs["b_comb"], np.float32).reshape(E, 1),
    }


def _prep_masks(mask):
    """mask: [B, G, N] f32 (0 / -1e9).  Returns fp8 pair-packed transposed
    mask (0/-240) and fp16 natural mask/10 (0/-24)."""
    import ml_dtypes

    neg = mask < 0
    # mdr[b, k, c, i, g] = M240[b, g, n=128c+2k+i]
    negT = neg.transpose(0, 2, 1)  # [B, N, G]
    negT = negT.reshape(mask.shape[0], 4, 64, 2, G).transpose(0, 2, 1, 3, 4)
    mdr = np.where(negT, np.float32(-240.0), np.float32(0.0)).astype(
        ml_dtypes.float8_e4m3fn
    )
    m16 = np.where(neg, np.float16(-24.0), np.float16(0.0))
    return np.ascontiguousarray(mdr), np.ascontiguousarray(m16)


def _make_in_maps(inputs):
    w = _prep_weights(inputs)
    nodes = np.ascontiguousarray(np.asarray(inputs["encoded_nodes"], np.float16))
    q1 = np.ascontiguousarray(np.asarray(inputs["encoded_q1"], np.float16))
    last = np.ascontiguousarray(np.asarray(inputs["encoded_last_node"], np.float16))
    mdr, m16 = _prep_masks(np.asarray(inputs["ninf_mask"], np.float32))

    in_maps = []
    for i in range(NCORES):
        sl = slice(i * BL, (i + 1) * BL)
        in_maps.append(
            {
                "nodes": nodes[sl],
                "q1": q1[sl],
                "last": last[sl],
                "mdr": mdr[sl],
                "m16": m16[sl],
                **w,
            }
        )
    return in_maps


def run(inputs, trace=False):
    from concourse.bass_utils import run_bass_kernel_spmd

    if "nc" not in _CACHE:
        _CACHE["nc"] = _build_nc()
    nc = _CACHE["nc"]

    in_maps = _make_in_maps(inputs)
    try:
        res = run_bass_kernel_spmd(nc, in_maps, list(range(NCORES)), trace=trace)
    except Exception:
        # The first execution of a freshly compiled NEFF occasionally dies
        # with NRT_EXEC_UNIT_UNRECOVERABLE on this stack; a retry with the
        # cached NEFF has always succeeded.
        res = run_bass_kernel_spmd(nc, in_maps, list(range(NCORES)), trace=trace)
    out = np.concatenate([res.results[i]["probs"] for i in range(NCORES)], axis=0)
    return out, res


def kernel(**inputs) -> np.ndarray:
    out, _ = run(inputs, trace=False)
    return out
